# revision 54
# baseline (speedup 1.0000x reference)
"""AttentiveRouter Trainium2 kernel.

Math: the reference computes
    q    = x @ Wq.T + bq                        [B,S,D]
    attn = softmax((q @ key_emb.T) / sqrt(D))   [B,S,E]
    scores = attn.mean(S); top-2 per batch; mask; router loss (KL + aux)

Everything up to softmax is linear in x, so fold the two matmuls:
    A = scale * (key_emb @ Wq).T          [D, E]   (tiny)
    c = scale * (key_emb @ bq)            [E]
    attn_logits = x @ A + c               [B,S,E]
This turns a 137 GFLOP problem into a ~1 GFLOP, HBM-bound one.

Sharding: data-parallel over batch B across 8 cores (8 batches/core);
Wq/bq/key_emb replicated; per-core partial aux sums + top-k indices are
combined on the host (the "all-reduce" of the scalar loss).

Precision: fp32 throughout. The 2nd/3rd expert score margin on the
reference inputs is ~1.2e-7 (fp64 ground truth), so bf16 anywhere in the
x @ A path flips top-k selections. fp32 PE matmuls keep score error ~1e-8.
"""

import numpy as np

B, S, D, E, K = 64, 1024, 1024, 8, 2
NCORES = 8
BPC = B // NCORES          # batches per core
T = BPC * S                # tokens per core
NB = T // 128              # 128-token blocks per core (64)
GRP = 4                    # token-blocks per group
NG = NB // GRP             # groups (16)
SCALE = float(D) ** -0.5
Z_COEF = 1e-3
AUX_COEF = 1e-3


def _build_nc():
    import concourse.bass as bass
    import concourse.bacc as bacc
    import concourse.tile as tile
    from concourse import mybir

    f32 = mybir.dt.float32
    i32 = mybir.dt.int32
    u32 = mybir.dt.uint32
    AF = mybir.ActivationFunctionType
    ALU = mybir.AluOpType
    PSUM = bass.MemorySpace.PSUM
    AX = mybir.AxisListType

    nc = bacc.Bacc(None, target_bir_lowering=False)
    x_d = nc.dram_tensor("x", [T, D], f32, kind="ExternalInput").ap()
    wq_d = nc.dram_tensor("wq", [D, D], f32, kind="ExternalInput").ap()
    bq_d = nc.dram_tensor("bq", [D], f32, kind="ExternalInput").ap()
    ke_d = nc.dram_tensor("ke", [E, D], f32, kind="ExternalInput").ap()
    mask_d = nc.dram_tensor("mask", [BPC, S, E], f32, kind="ExternalOutput").ap()
    idx_d = nc.dram_tensor("idx", [BPC, K], i32, kind="ExternalOutput").ap()
    aux_d = nc.dram_tensor("aux", [1, 1], f32, kind="ExternalOutput").ap()

    with tile.TileContext(nc) as tc:
        with (
            tc.tile_pool(name="const", bufs=1) as const,
            tc.tile_pool(name="xin", bufs=4) as xin,
            tc.tile_pool(name="xt", bufs=4) as xtp,
            tc.tile_pool(name="big", bufs=1) as big,
            tc.tile_pool(name="small", bufs=1) as small,
        ):
            # ---------- constants ----------
            id128 = const.tile([128, 128], f32)
            nc.vector.memset(id128, 1.0)
            nc.gpsimd.affine_select(
                out=id128, in_=id128, pattern=[[-1, 128]], base=0,
                channel_multiplier=1, compare_op=ALU.is_equal, fill=0.0,
            )
            id8 = const.tile([8, 8], f32)
            nc.vector.memset(id8, 1.0)
            nc.gpsimd.affine_select(
                out=id8, in_=id8, pattern=[[-1, 8]], base=0,
                channel_multiplier=1, compare_op=ALU.is_equal, fill=0.0,
            )
            ones_row = const.tile([1, 128], f32)   # K=1 lhsT for broadcasts
            nc.vector.memset(ones_row, 1.0)
            ones_col = const.tile([128, 1], f32)   # partition-sum rhs
            nc.vector.memset(ones_col, 1.0)
            iota_e = const.tile([BPC, E], f32)     # 0..7 along free, per row
            nc.gpsimd.iota(iota_e, pattern=[[1, E]], base=0, channel_multiplier=0,
                           allow_small_or_imprecise_dtypes=True)

            # ---- phase 0: A = SCALE*(key_emb @ Wq).T, c = SCALE*(key_emb @ bq)
            ke_sb = const.tile([E, D], f32)
            nc.sync.dma_start(out=ke_sb, in_=ke_d)
            bq_sb = const.tile([128, 8], f32)      # bq[c*128+p] at [p, c]
            nc.sync.dma_start(out=bq_sb, in_=bq_d.rearrange("(c p) -> p c", p=128))

            A_sb = [const.tile([128, 8], f32, tag=f"A{m}", name=f"A{m}")
                    for m in range(8)]
            keT_sb = [const.tile([128, 8], f32, tag=f"keT{c}", name=f"keT{c}")
                      for c in range(8)]
            c_sb = const.tile([1, 8], f32)

            with (
                tc.tile_pool(name="wq", bufs=1) as wqp,
                tc.tile_pool(name="ps0", bufs=2, space=PSUM) as ps0,
            ):
                wq_sb = []
                for c in range(8):                 # Wq rows chunk (contraction o)
                    w = wqp.tile([128, D], f32, tag=f"wq{c}", name=f"wqt{c}")
                    nc.sync.dma_start(out=w, in_=wq_d[c * 128:(c + 1) * 128, :])
                    wq_sb.append(w)

                for c in range(8):                 # key_emb.T chunks [128 o, 8 e]
                    pt = ps0.tile([128, 8], f32, tag="ps0", name=f"kps{c}")
                    nc.tensor.transpose(pt, ke_sb[:, c * 128:(c + 1) * 128], id8)
                    nc.vector.tensor_copy(keT_sb[c], pt)

                for m in range(8):                 # A chunks [128 d, 8 e], scaled
                    pa = ps0.tile([128, 8], f32, tag="ps0", name=f"aps{m}")
                    for c in range(8):
                        nc.tensor.matmul(
                            pa, wq_sb[c][:, m * 128:(m + 1) * 128], keT_sb[c],
                            start=(c == 0), stop=(c == 7),
                        )
                    nc.scalar.mul(out=A_sb[m], in_=pa, mul=SCALE)

                c_ps = ps0.tile([1, 8], f32, tag="ps0")
                for c in range(8):
                    nc.tensor.matmul(
                        c_ps, bq_sb[:, c:c + 1], keT_sb[c],
                        start=(c == 0), stop=(c == 7),
                    )
                nc.scalar.mul(out=c_sb, in_=c_ps, mul=SCALE)

            # ---------- phase 1: attn logits for all tokens ----------
            u_all = big.tile([128, NB * E], f32)    # exp(attn)  [p, (blk, e)]
            z_all = big.tile([128, NB * E], f32)    # attn       [p, (blk, e)]
            with (
                tc.tile_pool(name="ps_t", bufs=4, space=PSUM) as ps_t,
                tc.tile_pool(name="ps_attn", bufs=4, space=PSUM) as ps_attn,
            ):
                for g in range(NG):
                    xs = []
                    for b in range(GRP):
                        xt_in = xin.tile([128, D], f32, tag=f"x{b}")
                        blk = g * GRP + b
                        nc.sync.dma_start(
                            out=xt_in, in_=x_d[blk * 128:(blk + 1) * 128, :]
                        )
                        xs.append(xt_in)

                    xt_sb = []
                    for c in range(8):
                        pt = ps_t.tile([128, GRP * 128], f32, tag="xtps")
                        for b in range(GRP):
                            nc.tensor.transpose(
                                pt[:, b * 128:(b + 1) * 128],
                                xs[b][:, c * 128:(c + 1) * 128],
                                id128,
                            )
                        st = xtp.tile([128, GRP * 128], f32, tag=f"xt{c}")
                        # alternate copy engine so DVE and ACT share the load
                        if c % 2 == 0:
                            nc.vector.tensor_copy(st, pt)
                        else:
                            nc.scalar.copy(out=st, in_=pt)
                        xt_sb.append(st)

                    pattn = ps_attn.tile([128, GRP * E], f32, tag="attn")
                    for b in range(GRP):
                        osl = pattn[:, b * E:(b + 1) * E]
                        for c in range(8):
                            nc.tensor.matmul(
                                osl, xt_sb[c][:, b * 128:(b + 1) * 128], A_sb[c],
                                start=(c == 0), stop=False,
                            )
                        # + c broadcast over tokens (bq term; zero for given init)
                        nc.tensor.matmul(osl, ones_row, c_sb, start=False, stop=True)

                    sl = slice(g * GRP * E, (g + 1) * GRP * E)
                    nc.scalar.activation(out=u_all[:, sl], in_=pattn, func=AF.Exp)
                    nc.scalar.copy(out=z_all[:, sl], in_=pattn)

            # ---------- phase 2: softmax stats, scores, aux ----------
            with tc.tile_pool(name="ps_end", bufs=1, space=PSUM) as ps_end:
                Z_sb = small.tile([128, NB], f32)
                nc.vector.reduce_sum(
                    Z_sb, u_all[:].rearrange("p (b e) -> p b e", e=E), axis=AX.X
                )
                r_sb = small.tile([128, NB], f32)
                nc.vector.reciprocal(r_sb, Z_sb)

                # sum_t log Z per partition via activation accumulate
                logZ_waste = small.tile([128, NB], f32)
                aux2 = small.tile([128, 1], f32)
                nc.scalar.activation(
                    out=logZ_waste, in_=Z_sb, func=AF.Ln, accum_out=aux2
                )

                # sum_e u*z per token, then * r, then sum_t
                vz = big.tile([128, NB * E], f32)
                nc.vector.tensor_mul(vz, u_all, z_all)
                sdot = small.tile([128, NB], f32)
                nc.vector.reduce_sum(
                    sdot, vz[:].rearrange("p (b e) -> p b e", e=E), axis=AX.X
                )
                wz = small.tile([128, NB], f32)
                nc.vector.tensor_mul(wz, sdot, r_sb)
                aux1 = small.tile([128, 1], f32)
                nc.vector.reduce_sum(aux1, wz, axis=AX.X)
                auxp = small.tile([128, 1], f32)
                nc.vector.tensor_sub(auxp, aux1, aux2)
                aux_ps = ps_end.tile([1, 1], f32, tag="aux_ps")
                nc.tensor.matmul(aux_ps, auxp, ones_col, start=True, stop=True)
                aux_sb = small.tile([1, 1], f32)
                nc.vector.tensor_copy(aux_sb, aux_ps)
                nc.sync.dma_start(out=aux_d, in_=aux_sb)

                # scores[e, b] = sum_t u[t, e] * r[t]  (batch b = 8 blocks)
                sc_ps = ps_end.tile([8, BPC], f32, tag="sc_ps")
                for b in range(BPC):
                    for j in range(8):
                        blk = b * 8 + j
                        nc.tensor.matmul(
                            sc_ps[:, b:b + 1],
                            u_all[:, blk * E:(blk + 1) * E],
                            r_sb[:, blk:blk + 1],
                            start=(j == 0), stop=(j == 7),
                        )
                sc_eb = small.tile([8, BPC], f32)
                nc.vector.tensor_copy(sc_eb, sc_ps)
                scT_ps = ps_end.tile([BPC, 8], f32, tag="scT_ps")
                nc.tensor.transpose(scT_ps, sc_eb, id8)
                scT = small.tile([BPC, E], f32)
                nc.vector.tensor_copy(scT, scT_ps)

                # ---------- phase 3: top-2, indices, mask ----------
                mx8 = small.tile([BPC, 8], f32)
                nc.vector.max(out=mx8, in_=scT)
                ix8 = small.tile([BPC, 8], u32)
                nc.vector.max_index(out=ix8, in_max=mx8, in_values=scT)

                idx_sb = small.tile([BPC, K], i32)
                nc.vector.tensor_copy(idx_sb, ix8[:, 0:K])
                nc.sync.dma_start(out=idx_d, in_=idx_sb)

                ixf = small.tile([BPC, K], f32)
                nc.vector.tensor_copy(ixf, ix8[:, 0:K])
                sel1 = small.tile([BPC, E], f32)
                nc.vector.tensor_scalar(
                    sel1, iota_e, ixf[:, 0:1], None, op0=ALU.is_equal
                )
                sel2 = small.tile([BPC, E], f32)
                nc.vector.tensor_scalar(
                    sel2, iota_e, ixf[:, 1:2], None, op0=ALU.is_equal
                )
                m2d = small.tile([BPC, E], f32)
                nc.vector.tensor_add(m2d, sel1, sel2)

                # flatten [8, 8] -> [1, 64] across partitions (tiny DMA)
                m2f = small.tile([1, BPC * E], f32)
                nc.gpsimd.dma_start(out=m2f, in_=m2d)
                # repeat to [1, (b, si, e)] = [1, 512]
                m2rep = small.tile([1, BPC * 8 * E], f32)
                m2f_ap = m2f[:]
                src = bass.AP(
                    tensor=m2f_ap.tensor, offset=m2f_ap.offset,
                    ap=[m2f_ap.ap[0], [E, BPC], [0, 8], [1, E]],
                )
                nc.vector.tensor_copy(
                    m2rep[:].rearrange("p (b s e) -> p b s e", s=8, e=E), src
                )
                mk_ps = ps_end.tile([128, BPC * 8 * E], f32, tag="mk_ps")
                nc.tensor.matmul(mk_ps, ones_row, m2rep, start=True, stop=True)
                mk_sb = big.tile([128, BPC * 8 * E], f32)
                nc.vector.tensor_copy(mk_sb, mk_ps)
                # DRAM mask[b, s, e] with s = p*8 + si
                nc.sync.dma_start(
                    out=mask_d.rearrange("b (p s) e -> p b s e", p=128),
                    in_=mk_sb[:].rearrange("p (b s e) -> p b s e", s=8, e=E),
                )

    nc.compile()
    return nc


_NC_CACHE = None


def _get_nc():
    global _NC_CACHE
    if _NC_CACHE is None:
        _NC_CACHE = _build_nc()
    return _NC_CACHE


def _assemble(outs):
    mask = np.concatenate([o["mask"] for o in outs], axis=0)          # [B,S,E]
    idx = np.concatenate([o["idx"] for o in outs], axis=0)            # [B,K]
    aux_total = float(sum(float(np.asarray(o["aux"]).reshape(-1)[0]) for o in outs))

    counts = np.bincount(idx.ravel(), minlength=E).astype(np.float64)
    usage = counts * (S / float(B * S))
    ideal = 1.0 / E
    with np.errstate(divide="ignore"):
        kl = float(np.sum(ideal * (np.log(ideal) - np.log(usage))) / E)
    aux = aux_total / float(B * S * E)
    loss = np.float32(Z_COEF * kl + AUX_COEF * aux)
    return mask, idx.astype(np.int32), loss


def _in_maps(x, Wq, bq, key_emb):
    x = np.ascontiguousarray(x, dtype=np.float32)
    Wq = np.ascontiguousarray(Wq, dtype=np.float32)
    bq = np.ascontiguousarray(bq, dtype=np.float32)
    key_emb = np.ascontiguousarray(key_emb, dtype=np.float32)
    maps = []
    for core in range(NCORES):
        xs = np.ascontiguousarray(x[core * BPC:(core + 1) * BPC].reshape(T, D))
        maps.append({"x": xs, "wq": Wq, "bq": bq, "ke": key_emb})
    return maps


def kernel(x, Wq, bq, key_emb, _trace=False):
    from concourse import bass_utils

    nc = _get_nc()
    res = bass_utils.run_bass_kernel_spmd(
        nc, _in_maps(x, Wq, bq, key_emb),
        core_ids=list(range(NCORES)), trace=_trace,
    )
    out = _assemble(res.results)
    if _trace:
        return out, res
    return out


# revision 59
# speedup vs baseline: 1.0055x; 1.0055x over previous
"""AttentiveRouter Trainium2 kernel.

Math: the reference computes
    q    = x @ Wq.T + bq                        [B,S,D]
    attn = softmax((q @ key_emb.T) / sqrt(D))   [B,S,E]
    scores = attn.mean(S); top-2 per batch; mask; router loss (KL + aux)

Everything up to softmax is linear in x, so fold the two matmuls:
    A = scale * (key_emb @ Wq).T          [D, E]   (tiny)
    c = scale * (key_emb @ bq)            [E]
    attn_logits = x @ A + c               [B,S,E]
This turns a 137 GFLOP problem into a ~1 GFLOP, HBM-bound one.

Sharding: data-parallel over batch B across 8 cores (8 batches/core);
Wq/bq/key_emb replicated; per-core partial aux sums + top-k indices are
combined on the host (the "all-reduce" of the scalar loss).

Precision: fp32 throughout. The 2nd/3rd expert score margin on the
reference inputs is ~1.2e-7 (fp64 ground truth), so bf16 anywhere in the
x @ A path flips top-k selections. fp32 PE matmuls keep score error ~1e-8.
"""

import numpy as np

B, S, D, E, K = 64, 1024, 1024, 8, 2
NCORES = 8
BPC = B // NCORES          # batches per core
T = BPC * S                # tokens per core
NB = T // 128              # 128-token blocks per core (64)
GRP = 4                    # token-blocks per group
NG = NB // GRP             # groups (16)
SCALE = float(D) ** -0.5
Z_COEF = 1e-3
AUX_COEF = 1e-3


def _build_nc():
    import concourse.bass as bass
    import concourse.bacc as bacc
    import concourse.tile as tile
    from concourse import mybir

    f32 = mybir.dt.float32
    i32 = mybir.dt.int32
    u32 = mybir.dt.uint32
    AF = mybir.ActivationFunctionType
    ALU = mybir.AluOpType
    PSUM = bass.MemorySpace.PSUM
    AX = mybir.AxisListType

    nc = bacc.Bacc(None, target_bir_lowering=False)
    x_d = nc.dram_tensor("x", [T, D], f32, kind="ExternalInput").ap()
    wq_d = nc.dram_tensor("wq", [D, D], f32, kind="ExternalInput").ap()
    bq_d = nc.dram_tensor("bq", [D], f32, kind="ExternalInput").ap()
    ke_d = nc.dram_tensor("ke", [E, D], f32, kind="ExternalInput").ap()
    mask_d = nc.dram_tensor("mask", [BPC, S, E], f32, kind="ExternalOutput").ap()
    idx_d = nc.dram_tensor("idx", [BPC, K], i32, kind="ExternalOutput").ap()
    aux_d = nc.dram_tensor("aux", [1, 1], f32, kind="ExternalOutput").ap()

    with tile.TileContext(nc) as tc:
        with (
            tc.tile_pool(name="const", bufs=1) as const,
            tc.tile_pool(name="xin", bufs=4) as xin,
            tc.tile_pool(name="xt", bufs=4) as xtp,
            tc.tile_pool(name="big", bufs=1) as big,
            tc.tile_pool(name="small", bufs=1) as small,
        ):
            # ---------- constants ----------
            id128 = const.tile([128, 128], f32)
            nc.vector.memset(id128, 1.0)
            nc.gpsimd.affine_select(
                out=id128, in_=id128, pattern=[[-1, 128]], base=0,
                channel_multiplier=1, compare_op=ALU.is_equal, fill=0.0,
            )
            id8 = const.tile([8, 8], f32)
            nc.vector.memset(id8, 1.0)
            nc.gpsimd.affine_select(
                out=id8, in_=id8, pattern=[[-1, 8]], base=0,
                channel_multiplier=1, compare_op=ALU.is_equal, fill=0.0,
            )
            ones_row = const.tile([1, 128], f32)   # K=1 lhsT for broadcasts
            nc.vector.memset(ones_row, 1.0)
            ones_col = const.tile([128, 1], f32)   # partition-sum rhs
            nc.vector.memset(ones_col, 1.0)
            iota_e = const.tile([BPC, E], f32)     # 0..7 along free, per row
            nc.gpsimd.iota(iota_e, pattern=[[1, E]], base=0, channel_multiplier=0,
                           allow_small_or_imprecise_dtypes=True)

            # ---- phase 0: A = SCALE*(key_emb @ Wq).T, c = SCALE*(key_emb @ bq)
            ke_sb = const.tile([E, D], f32)
            nc.sync.dma_start(out=ke_sb, in_=ke_d)
            bq_sb = const.tile([128, 8], f32)      # bq[c*128+p] at [p, c]
            nc.sync.dma_start(out=bq_sb, in_=bq_d.rearrange("(c p) -> p c", p=128))

            A_sb = [const.tile([128, 8], f32, tag=f"A{m}", name=f"A{m}")
                    for m in range(8)]
            keT_sb = [const.tile([128, 8], f32, tag=f"keT{c}", name=f"keT{c}")
                      for c in range(8)]
            c_sb = const.tile([1, 8], f32)

            with (
                tc.tile_pool(name="wq", bufs=1) as wqp,
                tc.tile_pool(name="ps0", bufs=2, space=PSUM) as ps0,
            ):
                wq_sb = []
                for c in range(8):                 # Wq rows chunk (contraction o)
                    w = wqp.tile([128, D], f32, tag=f"wq{c}", name=f"wqt{c}")
                    nc.sync.dma_start(out=w, in_=wq_d[c * 128:(c + 1) * 128, :])
                    wq_sb.append(w)

                for c in range(8):                 # key_emb.T chunks [128 o, 8 e]
                    pt = ps0.tile([128, 8], f32, tag="ps0", name=f"kps{c}")
                    nc.tensor.transpose(pt, ke_sb[:, c * 128:(c + 1) * 128], id8)
                    nc.vector.tensor_copy(keT_sb[c], pt)

                for m in range(8):                 # A chunks [128 d, 8 e], scaled
                    pa = ps0.tile([128, 8], f32, tag="ps0", name=f"aps{m}")
                    for c in range(8):
                        nc.tensor.matmul(
                            pa, wq_sb[c][:, m * 128:(m + 1) * 128], keT_sb[c],
                            start=(c == 0), stop=(c == 7),
                        )
                    nc.scalar.mul(out=A_sb[m], in_=pa, mul=SCALE)

                c_ps = ps0.tile([1, 8], f32, tag="ps0")
                for c in range(8):
                    nc.tensor.matmul(
                        c_ps, bq_sb[:, c:c + 1], keT_sb[c],
                        start=(c == 0), stop=(c == 7),
                    )
                nc.scalar.mul(out=c_sb, in_=c_ps, mul=SCALE)

            # ---------- phase 1: attn logits for all tokens ----------
            u_all = big.tile([128, NB * E], f32)    # exp(attn)  [p, (blk, e)]
            z_all = big.tile([128, NB * E], f32)    # attn       [p, (blk, e)]
            with (
                tc.tile_pool(name="ps_t", bufs=6, space=PSUM) as ps_t,
                tc.tile_pool(name="ps_attn", bufs=2, space=PSUM) as ps_attn,
            ):
                for g in range(NG):
                    xs = []
                    for b in range(GRP):
                        xt_in = xin.tile([128, D], f32, tag=f"x{b}")
                        blk = g * GRP + b
                        nc.sync.dma_start(
                            out=xt_in, in_=x_d[blk * 128:(blk + 1) * 128, :]
                        )
                        xs.append(xt_in)

                    xt_sb = []
                    for c in range(8):
                        pt = ps_t.tile([128, GRP * 128], f32, tag="xtps")
                        for b in range(GRP):
                            nc.tensor.transpose(
                                pt[:, b * 128:(b + 1) * 128],
                                xs[b][:, c * 128:(c + 1) * 128],
                                id128,
                            )
                        st = xtp.tile([128, GRP * 128], f32, tag=f"xt{c}")
                        # alternate copy engine so DVE and ACT share the load
                        if c % 2 == 0:
                            nc.vector.tensor_copy(st, pt)
                        else:
                            nc.scalar.copy(out=st, in_=pt)
                        xt_sb.append(st)

                    pattn = ps_attn.tile([128, GRP * E], f32, tag="attn")
                    for b in range(GRP):
                        osl = pattn[:, b * E:(b + 1) * E]
                        for c in range(8):
                            nc.tensor.matmul(
                                osl, xt_sb[c][:, b * 128:(b + 1) * 128], A_sb[c],
                                start=(c == 0), stop=False,
                            )
                        # + c broadcast over tokens (bq term; zero for given init)
                        nc.tensor.matmul(osl, ones_row, c_sb, start=False, stop=True)

                    sl = slice(g * GRP * E, (g + 1) * GRP * E)
                    nc.scalar.activation(out=u_all[:, sl], in_=pattn, func=AF.Exp)
                    nc.scalar.copy(out=z_all[:, sl], in_=pattn)

            # ---------- phase 2: softmax stats, scores, aux ----------
            with tc.tile_pool(name="ps_end", bufs=1, space=PSUM) as ps_end:
                Z_sb = small.tile([128, NB], f32)
                nc.vector.reduce_sum(
                    Z_sb, u_all[:].rearrange("p (b e) -> p b e", e=E), axis=AX.X
                )
                r_sb = small.tile([128, NB], f32)
                nc.vector.reciprocal(r_sb, Z_sb)

                # sum_t log Z per partition via activation accumulate
                logZ_waste = small.tile([128, NB], f32)
                aux2 = small.tile([128, 1], f32)
                nc.scalar.activation(
                    out=logZ_waste, in_=Z_sb, func=AF.Ln, accum_out=aux2
                )

                # sum_e u*z per token, then * r, then sum_t
                vz = big.tile([128, NB * E], f32)
                nc.vector.tensor_mul(vz, u_all, z_all)
                sdot = small.tile([128, NB], f32)
                nc.vector.reduce_sum(
                    sdot, vz[:].rearrange("p (b e) -> p b e", e=E), axis=AX.X
                )
                wz = small.tile([128, NB], f32)
                nc.vector.tensor_mul(wz, sdot, r_sb)
                aux1 = small.tile([128, 1], f32)
                nc.vector.reduce_sum(aux1, wz, axis=AX.X)
                auxp = small.tile([128, 1], f32)
                nc.vector.tensor_sub(auxp, aux1, aux2)
                aux_ps = ps_end.tile([1, 1], f32, tag="aux_ps")
                nc.tensor.matmul(aux_ps, auxp, ones_col, start=True, stop=True)
                aux_sb = small.tile([1, 1], f32)
                nc.vector.tensor_copy(aux_sb, aux_ps)
                nc.sync.dma_start(out=aux_d, in_=aux_sb)

                # scores[e, b] = sum_t u[t, e] * r[t]  (batch b = 8 blocks)
                sc_ps = ps_end.tile([8, BPC], f32, tag="sc_ps")
                for b in range(BPC):
                    for j in range(8):
                        blk = b * 8 + j
                        nc.tensor.matmul(
                            sc_ps[:, b:b + 1],
                            u_all[:, blk * E:(blk + 1) * E],
                            r_sb[:, blk:blk + 1],
                            start=(j == 0), stop=(j == 7),
                        )
                sc_eb = small.tile([8, BPC], f32)
                nc.vector.tensor_copy(sc_eb, sc_ps)
                scT_ps = ps_end.tile([BPC, 8], f32, tag="scT_ps")
                nc.tensor.transpose(scT_ps, sc_eb, id8)
                scT = small.tile([BPC, E], f32)
                nc.vector.tensor_copy(scT, scT_ps)

                # ---------- phase 3: top-2, indices, mask ----------
                mx8 = small.tile([BPC, 8], f32)
                nc.vector.max(out=mx8, in_=scT)
                ix8 = small.tile([BPC, 8], u32)
                nc.vector.max_index(out=ix8, in_max=mx8, in_values=scT)

                idx_sb = small.tile([BPC, K], i32)
                nc.vector.tensor_copy(idx_sb, ix8[:, 0:K])
                nc.sync.dma_start(out=idx_d, in_=idx_sb)

                ixf = small.tile([BPC, K], f32)
                nc.vector.tensor_copy(ixf, ix8[:, 0:K])
                sel1 = small.tile([BPC, E], f32)
                nc.vector.tensor_scalar(
                    sel1, iota_e, ixf[:, 0:1], None, op0=ALU.is_equal
                )
                sel2 = small.tile([BPC, E], f32)
                nc.vector.tensor_scalar(
                    sel2, iota_e, ixf[:, 1:2], None, op0=ALU.is_equal
                )
                m2d = small.tile([BPC, E], f32)
                nc.vector.tensor_add(m2d, sel1, sel2)

                # flatten [8, 8] -> [1, 64] across partitions (tiny DMA)
                m2f = small.tile([1, BPC * E], f32)
                nc.gpsimd.dma_start(out=m2f, in_=m2d)
                # repeat to [1, (b, si, e)] = [1, 512]
                m2rep = small.tile([1, BPC * 8 * E], f32)
                m2f_ap = m2f[:]
                src = bass.AP(
                    tensor=m2f_ap.tensor, offset=m2f_ap.offset,
                    ap=[m2f_ap.ap[0], [E, BPC], [0, 8], [1, E]],
                )
                nc.vector.tensor_copy(
                    m2rep[:].rearrange("p (b s e) -> p b s e", s=8, e=E), src
                )
                mk_ps = ps_end.tile([128, BPC * 8 * E], f32, tag="mk_ps")
                nc.tensor.matmul(mk_ps, ones_row, m2rep, start=True, stop=True)
                mk_sb = big.tile([128, BPC * 8 * E], f32)
                nc.vector.tensor_copy(mk_sb, mk_ps)
                # DRAM mask[b, s, e] with s = p*8 + si
                nc.sync.dma_start(
                    out=mask_d.rearrange("b (p s) e -> p b s e", p=128),
                    in_=mk_sb[:].rearrange("p (b s e) -> p b s e", s=8, e=E),
                )

    nc.compile()
    return nc


_NC_CACHE = None


def _get_nc():
    global _NC_CACHE
    if _NC_CACHE is None:
        _NC_CACHE = _build_nc()
    return _NC_CACHE


def _assemble(outs):
    mask = np.concatenate([o["mask"] for o in outs], axis=0)          # [B,S,E]
    idx = np.concatenate([o["idx"] for o in outs], axis=0)            # [B,K]
    aux_total = float(sum(float(np.asarray(o["aux"]).reshape(-1)[0]) for o in outs))

    counts = np.bincount(idx.ravel(), minlength=E).astype(np.float64)
    usage = counts * (S / float(B * S))
    ideal = 1.0 / E
    with np.errstate(divide="ignore"):
        kl = float(np.sum(ideal * (np.log(ideal) - np.log(usage))) / E)
    aux = aux_total / float(B * S * E)
    loss = np.float32(Z_COEF * kl + AUX_COEF * aux)
    return mask, idx.astype(np.int32), loss


def _in_maps(x, Wq, bq, key_emb):
    x = np.ascontiguousarray(x, dtype=np.float32)
    Wq = np.ascontiguousarray(Wq, dtype=np.float32)
    bq = np.ascontiguousarray(bq, dtype=np.float32)
    key_emb = np.ascontiguousarray(key_emb, dtype=np.float32)
    maps = []
    for core in range(NCORES):
        xs = np.ascontiguousarray(x[core * BPC:(core + 1) * BPC].reshape(T, D))
        maps.append({"x": xs, "wq": Wq, "bq": bq, "ke": key_emb})
    return maps


def kernel(x, Wq, bq, key_emb, _trace=False):
    from concourse import bass_utils

    nc = _get_nc()
    res = bass_utils.run_bass_kernel_spmd(
        nc, _in_maps(x, Wq, bq, key_emb),
        core_ids=list(range(NCORES)), trace=_trace,
    )
    out = _assemble(res.results)
    if _trace:
        return out, res
    return out


# revision 62
# speedup vs baseline: 1.0113x; 1.0058x over previous
"""AttentiveRouter Trainium2 kernel.

Math: the reference computes
    q    = x @ Wq.T + bq                        [B,S,D]
    attn = softmax((q @ key_emb.T) / sqrt(D))   [B,S,E]
    scores = attn.mean(S); top-2 per batch; mask; router loss (KL + aux)

Everything up to softmax is linear in x, so fold the two matmuls:
    A = scale * (key_emb @ Wq).T          [D, E]   (tiny)
    c = scale * (key_emb @ bq)            [E]
    attn_logits = x @ A + c               [B,S,E]
This turns a 137 GFLOP problem into a ~1 GFLOP, HBM-bound one.

Sharding: data-parallel over batch B across 8 cores (8 batches/core);
Wq/bq/key_emb replicated; per-core partial aux sums + top-k indices are
combined on the host (the "all-reduce" of the scalar loss).

Precision: fp32 throughout. The 2nd/3rd expert score margin on the
reference inputs is ~1.2e-7 (fp64 ground truth), so bf16 anywhere in the
x @ A path flips top-k selections. fp32 PE matmuls keep score error ~1e-8.
"""

import numpy as np

B, S, D, E, K = 64, 1024, 1024, 8, 2
NCORES = 8
BPC = B // NCORES          # batches per core
T = BPC * S                # tokens per core
NB = T // 128              # 128-token blocks per core (64)
GRP = 4                    # token-blocks per group
NG = NB // GRP             # groups (16)
SCALE = float(D) ** -0.5
Z_COEF = 1e-3
AUX_COEF = 1e-3


def _build_nc():
    import concourse.bass as bass
    import concourse.bacc as bacc
    import concourse.tile as tile
    from concourse import mybir

    f32 = mybir.dt.float32
    i32 = mybir.dt.int32
    u32 = mybir.dt.uint32
    AF = mybir.ActivationFunctionType
    ALU = mybir.AluOpType
    PSUM = bass.MemorySpace.PSUM
    AX = mybir.AxisListType

    nc = bacc.Bacc(None, target_bir_lowering=False)
    x_d = nc.dram_tensor("x", [T, D], f32, kind="ExternalInput").ap()
    wq_d = nc.dram_tensor("wq", [D, D], f32, kind="ExternalInput").ap()
    bq_d = nc.dram_tensor("bq", [D], f32, kind="ExternalInput").ap()
    ke_d = nc.dram_tensor("ke", [E, D], f32, kind="ExternalInput").ap()
    mask_d = nc.dram_tensor("mask", [BPC, S, E], f32, kind="ExternalOutput").ap()
    idx_d = nc.dram_tensor("idx", [BPC, K], i32, kind="ExternalOutput").ap()
    aux_d = nc.dram_tensor("aux", [1, 1], f32, kind="ExternalOutput").ap()

    with tile.TileContext(nc) as tc:
        with (
            tc.tile_pool(name="const", bufs=1) as const,
            tc.tile_pool(name="xin", bufs=4) as xin,
            tc.tile_pool(name="xt", bufs=4) as xtp,
            tc.tile_pool(name="big", bufs=1) as big,
            tc.tile_pool(name="small", bufs=1) as small,
        ):
            # ---------- constants ----------
            id128 = const.tile([128, 128], f32)
            nc.vector.memset(id128, 1.0)
            nc.gpsimd.affine_select(
                out=id128, in_=id128, pattern=[[-1, 128]], base=0,
                channel_multiplier=1, compare_op=ALU.is_equal, fill=0.0,
            )
            id8 = const.tile([8, 8], f32)
            nc.vector.memset(id8, 1.0)
            nc.gpsimd.affine_select(
                out=id8, in_=id8, pattern=[[-1, 8]], base=0,
                channel_multiplier=1, compare_op=ALU.is_equal, fill=0.0,
            )
            ones_row = const.tile([1, 128], f32)   # K=1 lhsT for broadcasts
            nc.vector.memset(ones_row, 1.0)
            ones_col = const.tile([128, 1], f32)   # partition-sum rhs
            nc.vector.memset(ones_col, 1.0)
            iota_e = const.tile([BPC, E], f32)     # 0..7 along free, per row
            nc.gpsimd.iota(iota_e, pattern=[[1, E]], base=0, channel_multiplier=0,
                           allow_small_or_imprecise_dtypes=True)

            # ---- phase 0: A = SCALE*(key_emb @ Wq).T, c = SCALE*(key_emb @ bq)
            ke_sb = const.tile([E, D], f32)
            nc.sync.dma_start(out=ke_sb, in_=ke_d)
            bq_sb = const.tile([128, 8], f32)      # bq[c*128+p] at [p, c]
            nc.sync.dma_start(out=bq_sb, in_=bq_d.rearrange("(c p) -> p c", p=128))

            A_sb = [const.tile([128, 8], f32, tag=f"A{m}", name=f"A{m}")
                    for m in range(8)]
            keT_sb = [const.tile([128, 8], f32, tag=f"keT{c}", name=f"keT{c}")
                      for c in range(8)]
            c_sb = const.tile([1, 8], f32)

            with (
                tc.tile_pool(name="wq", bufs=1) as wqp,
                tc.tile_pool(name="ps0", bufs=2, space=PSUM) as ps0,
            ):
                wq_sb = []
                for c in range(8):                 # Wq rows chunk (contraction o)
                    w = wqp.tile([128, D], f32, tag=f"wq{c}", name=f"wqt{c}")
                    nc.sync.dma_start(out=w, in_=wq_d[c * 128:(c + 1) * 128, :])
                    wq_sb.append(w)

                for c in range(8):                 # key_emb.T chunks [128 o, 8 e]
                    pt = ps0.tile([128, 8], f32, tag="ps0", name=f"kps{c}")
                    nc.tensor.transpose(pt, ke_sb[:, c * 128:(c + 1) * 128], id8)
                    nc.vector.tensor_copy(keT_sb[c], pt)

                for m in range(8):                 # A chunks [128 d, 8 e], scaled
                    pa = ps0.tile([128, 8], f32, tag="ps0", name=f"aps{m}")
                    for c in range(8):
                        nc.tensor.matmul(
                            pa, wq_sb[c][:, m * 128:(m + 1) * 128], keT_sb[c],
                            start=(c == 0), stop=(c == 7),
                        )
                    nc.scalar.mul(out=A_sb[m], in_=pa, mul=SCALE)

                c_ps = ps0.tile([1, 8], f32, tag="ps0")
                for c in range(8):
                    nc.tensor.matmul(
                        c_ps, bq_sb[:, c:c + 1], keT_sb[c],
                        start=(c == 0), stop=(c == 7),
                    )
                nc.scalar.mul(out=c_sb, in_=c_ps, mul=SCALE)

            # ---------- phase 1: attn logits for all tokens ----------
            u_all = big.tile([128, NB * E], f32)    # exp(attn)  [p, (blk, e)]
            z_all = big.tile([128, NB * E], f32)    # attn       [p, (blk, e)]
            with (
                tc.tile_pool(name="ps_t", bufs=6, space=PSUM) as ps_t,
                tc.tile_pool(name="ps_attn", bufs=2, space=PSUM) as ps_attn,
            ):
                for g in range(NG):
                    xs = []
                    for b in range(GRP):
                        xt_in = xin.tile([128, D], f32, tag=f"x{b}")
                        blk = g * GRP + b
                        rows = x_d[blk * 128:(blk + 1) * 128, :]
                        nc.sync.dma_start(out=xt_in[:, 0:D // 2],
                                          in_=rows[:, 0:D // 2])
                        nc.sync.dma_start(out=xt_in[:, D // 2:D],
                                          in_=rows[:, D // 2:D])
                        xs.append(xt_in)

                    xt_sb = []
                    for c in range(8):
                        pt = ps_t.tile([128, GRP * 128], f32, tag="xtps")
                        for b in range(GRP):
                            nc.tensor.transpose(
                                pt[:, b * 128:(b + 1) * 128],
                                xs[b][:, c * 128:(c + 1) * 128],
                                id128,
                            )
                        st = xtp.tile([128, GRP * 128], f32, tag=f"xt{c}")
                        # alternate copy engine so DVE and ACT share the load
                        if c % 2 == 0:
                            nc.vector.tensor_copy(st, pt)
                        else:
                            nc.scalar.copy(out=st, in_=pt)
                        xt_sb.append(st)

                    pattn = ps_attn.tile([128, GRP * E], f32, tag="attn")
                    for b in range(GRP):
                        osl = pattn[:, b * E:(b + 1) * E]
                        for c in range(8):
                            nc.tensor.matmul(
                                osl, xt_sb[c][:, b * 128:(b + 1) * 128], A_sb[c],
                                start=(c == 0), stop=False,
                            )
                        # + c broadcast over tokens (bq term; zero for given init)
                        nc.tensor.matmul(osl, ones_row, c_sb, start=False, stop=True)

                    sl = slice(g * GRP * E, (g + 1) * GRP * E)
                    nc.scalar.activation(out=u_all[:, sl], in_=pattn, func=AF.Exp)
                    nc.scalar.copy(out=z_all[:, sl], in_=pattn)

            # ---------- phase 2: softmax stats, scores, aux ----------
            with tc.tile_pool(name="ps_end", bufs=1, space=PSUM) as ps_end:
                Z_sb = small.tile([128, NB], f32)
                nc.vector.reduce_sum(
                    Z_sb, u_all[:].rearrange("p (b e) -> p b e", e=E), axis=AX.X
                )
                r_sb = small.tile([128, NB], f32)
                nc.vector.reciprocal(r_sb, Z_sb)

                # sum_t log Z per partition via activation accumulate
                logZ_waste = small.tile([128, NB], f32)
                aux2 = small.tile([128, 1], f32)
                nc.scalar.activation(
                    out=logZ_waste, in_=Z_sb, func=AF.Ln, accum_out=aux2
                )

                # sum_e u*z per token, then * r, then sum_t
                vz = big.tile([128, NB * E], f32)
                nc.vector.tensor_mul(vz, u_all, z_all)
                sdot = small.tile([128, NB], f32)
                nc.vector.reduce_sum(
                    sdot, vz[:].rearrange("p (b e) -> p b e", e=E), axis=AX.X
                )
                wz = small.tile([128, NB], f32)
                nc.vector.tensor_mul(wz, sdot, r_sb)
                aux1 = small.tile([128, 1], f32)
                nc.vector.reduce_sum(aux1, wz, axis=AX.X)
                auxp = small.tile([128, 1], f32)
                nc.vector.tensor_sub(auxp, aux1, aux2)
                aux_ps = ps_end.tile([1, 1], f32, tag="aux_ps")
                nc.tensor.matmul(aux_ps, auxp, ones_col, start=True, stop=True)
                aux_sb = small.tile([1, 1], f32)
                nc.vector.tensor_copy(aux_sb, aux_ps)
                nc.sync.dma_start(out=aux_d, in_=aux_sb)

                # scores[e, b] = sum_t u[t, e] * r[t]  (batch b = 8 blocks)
                sc_ps = ps_end.tile([8, BPC], f32, tag="sc_ps")
                for b in range(BPC):
                    for j in range(8):
                        blk = b * 8 + j
                        nc.tensor.matmul(
                            sc_ps[:, b:b + 1],
                            u_all[:, blk * E:(blk + 1) * E],
                            r_sb[:, blk:blk + 1],
                            start=(j == 0), stop=(j == 7),
                        )
                sc_eb = small.tile([8, BPC], f32)
                nc.vector.tensor_copy(sc_eb, sc_ps)
                scT_ps = ps_end.tile([BPC, 8], f32, tag="scT_ps")
                nc.tensor.transpose(scT_ps, sc_eb, id8)
                scT = small.tile([BPC, E], f32)
                nc.vector.tensor_copy(scT, scT_ps)

                # ---------- phase 3: top-2, indices, mask ----------
                mx8 = small.tile([BPC, 8], f32)
                nc.vector.max(out=mx8, in_=scT)
                ix8 = small.tile([BPC, 8], u32)
                nc.vector.max_index(out=ix8, in_max=mx8, in_values=scT)

                idx_sb = small.tile([BPC, K], i32)
                nc.vector.tensor_copy(idx_sb, ix8[:, 0:K])
                nc.sync.dma_start(out=idx_d, in_=idx_sb)

                ixf = small.tile([BPC, K], f32)
                nc.vector.tensor_copy(ixf, ix8[:, 0:K])
                sel1 = small.tile([BPC, E], f32)
                nc.vector.tensor_scalar(
                    sel1, iota_e, ixf[:, 0:1], None, op0=ALU.is_equal
                )
                sel2 = small.tile([BPC, E], f32)
                nc.vector.tensor_scalar(
                    sel2, iota_e, ixf[:, 1:2], None, op0=ALU.is_equal
                )
                m2d = small.tile([BPC, E], f32)
                nc.vector.tensor_add(m2d, sel1, sel2)

                # flatten [8, 8] -> [1, 64] across partitions (tiny DMA)
                m2f = small.tile([1, BPC * E], f32)
                nc.gpsimd.dma_start(out=m2f, in_=m2d)
                # repeat to [1, (b, si, e)] = [1, 512]
                m2rep = small.tile([1, BPC * 8 * E], f32)
                m2f_ap = m2f[:]
                src = bass.AP(
                    tensor=m2f_ap.tensor, offset=m2f_ap.offset,
                    ap=[m2f_ap.ap[0], [E, BPC], [0, 8], [1, E]],
                )
                nc.vector.tensor_copy(
                    m2rep[:].rearrange("p (b s e) -> p b s e", s=8, e=E), src
                )
                mk_ps = ps_end.tile([128, BPC * 8 * E], f32, tag="mk_ps")
                nc.tensor.matmul(mk_ps, ones_row, m2rep, start=True, stop=True)
                mk_sb = big.tile([128, BPC * 8 * E], f32)
                nc.vector.tensor_copy(mk_sb, mk_ps)
                # DRAM mask[b, s, e] with s = p*8 + si
                nc.sync.dma_start(
                    out=mask_d.rearrange("b (p s) e -> p b s e", p=128),
                    in_=mk_sb[:].rearrange("p (b s e) -> p b s e", s=8, e=E),
                )

    nc.compile()
    return nc


_NC_CACHE = None


def _get_nc():
    global _NC_CACHE
    if _NC_CACHE is None:
        _NC_CACHE = _build_nc()
    return _NC_CACHE


def _assemble(outs):
    mask = np.concatenate([o["mask"] for o in outs], axis=0)          # [B,S,E]
    idx = np.concatenate([o["idx"] for o in outs], axis=0)            # [B,K]
    aux_total = float(sum(float(np.asarray(o["aux"]).reshape(-1)[0]) for o in outs))

    counts = np.bincount(idx.ravel(), minlength=E).astype(np.float64)
    usage = counts * (S / float(B * S))
    ideal = 1.0 / E
    with np.errstate(divide="ignore"):
        kl = float(np.sum(ideal * (np.log(ideal) - np.log(usage))) / E)
    aux = aux_total / float(B * S * E)
    loss = np.float32(Z_COEF * kl + AUX_COEF * aux)
    return mask, idx.astype(np.int32), loss


def _in_maps(x, Wq, bq, key_emb):
    x = np.ascontiguousarray(x, dtype=np.float32)
    Wq = np.ascontiguousarray(Wq, dtype=np.float32)
    bq = np.ascontiguousarray(bq, dtype=np.float32)
    key_emb = np.ascontiguousarray(key_emb, dtype=np.float32)
    maps = []
    for core in range(NCORES):
        xs = np.ascontiguousarray(x[core * BPC:(core + 1) * BPC].reshape(T, D))
        maps.append({"x": xs, "wq": Wq, "bq": bq, "ke": key_emb})
    return maps


def kernel(x, Wq, bq, key_emb, _trace=False):
    from concourse import bass_utils

    nc = _get_nc()
    res = bass_utils.run_bass_kernel_spmd(
        nc, _in_maps(x, Wq, bq, key_emb),
        core_ids=list(range(NCORES)), trace=_trace,
    )
    out = _assemble(res.results)
    if _trace:
        return out, res
    return out
